# revision 4
# baseline (speedup 1.0000x reference)
"""MoE conv block (top-2 routed 3x3 conv experts) on 8 Trainium2 NeuronCores.

Strategy:
  - Router (global-avg-pool -> linear gate -> softmax -> top-2 -> renormalize)
    is tiny (~8 MFLOP) and runs on host in fp32, mirroring the reference math.
  - Convolution is linear in the weights, so the top-2 expert convs per sample
    collapse into ONE conv with per-sample combined weights
        W_b = w1 * W_e1 + w2 * W_e2,  b_b = w1 * b_e1 + w2 * b_e2.
  - Data-parallel over batch: 4 samples per core x 8 cores. Each core runs
    4 convs of [256,64,64] x [256,256,3,3] (SAME padding).
  - On device the 3x3 conv is 18 accumulated bf16 matmuls per PSUM tile
    (2 ci-chunks x 9 taps), K=128, M=128, N=512 (8 output rows x 64 cols),
    using a zero-padded [66,66] input layout in SBUF so spatial shifts are
    plain strided access patterns. PSUM accumulates in fp32; bias is added
    during the PSUM->SBUF eviction on the vector engine.
  - x is loaded/cast in row-halves and compute is ordered row-group-major so
    the tensor engine starts ~half an x-transfer earlier; a short warmup
    matmul burst during the prologue lifts the PE clock gate (HAM) to 2.4GHz
    before the real stream begins.
"""

import time

import numpy as np
import ml_dtypes

B, C, H, W = 32, 256, 64, 64
E, TOP_K = 8, 2
N_CORES = 8
BPC = B // N_CORES  # samples per core
HP = H + 2  # zero-padded spatial extent
HSPLIT = 33  # x rows 0..32 -> padded rows 1..33 (row-group 0 needs padded 0..33)

_COMPILED = None


def _build():
    """Build + compile the per-core Bass kernel (cached)."""
    global _COMPILED
    if _COMPILED is not None:
        return _COMPILED

    import concourse.bacc as bacc
    import concourse.tile as tile
    from concourse import mybir

    f32 = mybir.dt.float32
    bf16 = mybir.dt.bfloat16

    nc = bacc.Bacc("TRN2", target_bir_lowering=False, debug=False)
    x_d = nc.dram_tensor("x", [BPC, C, H, W], f32, kind="ExternalInput").ap()
    w_d = nc.dram_tensor("w", [BPC, C, 9, C], bf16, kind="ExternalInput").ap()
    b_d = nc.dram_tensor("bias", [BPC, C, 1], f32, kind="ExternalInput").ap()
    o_d = nc.dram_tensor("out", [BPC, C, H, W], f32, kind="ExternalOutput").ap()

    with tile.TileContext(nc) as tc:
        with (
            tc.tile_pool(name="warmp", bufs=1) as warm_pool,
            tc.tile_pool(name="xpadp", bufs=1) as xpad_pool,
            tc.tile_pool(name="stagep", bufs=4) as stage_pool,
            tc.tile_pool(name="wtp", bufs=2) as wt_pool,
            tc.tile_pool(name="biasp", bufs=2) as bias_pool,
            tc.tile_pool(name="outp", bufs=4) as out_pool,
            tc.tile_pool(name="psump", bufs=8, space="PSUM") as psum_pool,
        ):
            # --- PE warmup: ~3.5us of matmuls on zeroed data lifts the HAM
            # clock gate to 2.4GHz while the first DMAs are in flight.
            wz = warm_pool.tile([128, 512], bf16, name="wz")
            nc.vector.memset(wz[:], 0.0)
            wps = psum_pool.tile([128, 8, W], f32, name="ps")
            for i in range(16):
                nc.tensor.matmul(wps[:], wz[:, 0:128], wz[:],
                                 start=(i == 0), stop=(i == 15))
            wsink = warm_pool.tile([128, 1], f32, name="wsink")
            nc.vector.reduce_max(wsink[:], wps[:], axis=mybir.AxisListType.XY)

            # Two persistent zero-padded input buffers (double buffer across
            # samples). Borders are zeroed once; per-sample writes only touch
            # the interior, so the zero border persists.
            xpads = []
            for i in range(2):
                xp = xpad_pool.tile([128, 2, HP, HP], bf16, name=f"xpad{i}")
                nc.vector.memset(xp[:], 0.0)
                xpads.append(xp)

            for b in range(BPC):
                xp = xpads[b % 2]

                # x (fp32) arrives in row-halves and is cast into the padded
                # bf16 interior. Top halves (needed by row-group 0) are issued
                # and cast first so the tensor engine can start earliest.
                halves = {}
                for (r0, r1) in ((0, HSPLIT), (HSPLIT, H)):
                    for ch in range(2):
                        st = stage_pool.tile([128, HSPLIT, W], f32, name="stage")
                        nr = r1 - r0
                        nc.sync.dma_start(
                            st[:, :nr, :], x_d[b, ch * 128:(ch + 1) * 128, r0:r1, :])
                        halves[(ch, r0)] = (st, nr)

                # Combined conv weights, laid out [ci, tap, co] in bf16.
                wt = wt_pool.tile([128, 2, 9, C], bf16, name="wt")
                for ch in range(2):
                    nc.sync.dma_start(wt[:, ch], w_d[b, ch * 128:(ch + 1) * 128, :, :])

                bt = bias_pool.tile([128, 2], f32, name="bt")
                for co in range(2):
                    nc.sync.dma_start(bt[:, co:co + 1], b_d[b, co * 128:(co + 1) * 128, :])

                for (r0, r1) in ((0, HSPLIT), (HSPLIT, H)):
                    for ch in range(2):
                        st, nr = halves[(ch, r0)]
                        nc.vector.tensor_copy(
                            xp[:, ch, 1 + r0:1 + r0 + nr, 1:W + 1], st[:, :nr, :])

                for rg in range(2):  # row-groups of 4 row-blocks (4 PSUM banks)
                    for co in range(2):
                        ot = out_pool.tile([128, 32, W], f32, name="ot")
                        ps = [
                            psum_pool.tile([128, 8, W], f32, name="ps")
                            for _ in range(4)
                        ]
                        n = 0
                        for ch in range(2):
                            for kk in range(9):
                                ky, kx = kk // 3, kk % 3
                                lhsT = wt[:, ch, kk, co * 128:(co + 1) * 128]
                                for j in range(4):
                                    h0 = rg * 32 + j * 8
                                    rhs = xp[:, ch, h0 + ky:h0 + ky + 8, kx:kx + W]
                                    nc.tensor.matmul(
                                        ps[j][:], lhsT, rhs,
                                        start=(n == 0), stop=(n == 17),
                                    )
                                n += 1
                        for j in range(4):
                            nc.vector.tensor_scalar_add(
                                ot[:, j * 8:(j + 1) * 8, :], ps[j][:], bt[:, co:co + 1])
                        # outputs go out on the second HWDGE ring (ACT engine)
                        # so stores never queue ahead of loads on the SP ring
                        nc.scalar.dma_start(
                            o_d[b, co * 128:(co + 1) * 128, rg * 32:(rg + 1) * 32, :],
                            ot[:])

    nc.compile()
    _COMPILED = nc
    return nc


def _route_and_combine(x, gate_w, gate_b, conv_w, conv_b):
    """Host-side router (mirrors reference) + per-sample weight combination."""
    xf = np.asarray(x, dtype=np.float32)
    pooled = xf.mean(axis=(2, 3))                                # [B, C]
    logits = pooled @ np.asarray(gate_w, np.float32).T + np.asarray(gate_b, np.float32)
    z = logits - logits.max(-1, keepdims=True)
    wgt = np.exp(z)
    wgt /= wgt.sum(-1, keepdims=True)                            # softmax [B, E]
    top_i = np.argsort(-wgt, axis=-1, kind="stable")[:, :TOP_K]  # [B, K]
    top_w = np.take_along_axis(wgt, top_i, axis=-1)
    tz = top_w - top_w.max(-1, keepdims=True)
    tw = np.exp(tz)
    tw /= tw.sum(-1, keepdims=True)                              # renormalized [B, K]

    cw = np.asarray(conv_w, np.float32)                          # [E, co, ci, 3, 3]
    cb = np.asarray(conv_b, np.float32)                          # [E, co]
    Wc = (cw[top_i[:, 0]] * tw[:, 0, None, None, None, None]
          + cw[top_i[:, 1]] * tw[:, 1, None, None, None, None])  # [B, co, ci, 3, 3]
    bc = cb[top_i[:, 0]] * tw[:, 0, None] + cb[top_i[:, 1]] * tw[:, 1, None]  # [B, co]

    # Device layout: [b, ci, tap(ky*3+kx), co] bf16 — stationary operand slices
    # [K=ci(128), M=co(128)] become contiguous reads.
    Wd = np.ascontiguousarray(Wc.transpose(0, 2, 3, 4, 1)).reshape(B, C, 9, C)
    Wd = Wd.astype(ml_dtypes.bfloat16)
    return xf, Wd, bc.reshape(B, C, 1).astype(np.float32)


def run_sharded(inputs, trace=False, trace_cores=None):
    """Shard, run the SPMD bass kernel on 8 cores, gather. Returns
    (full_output, BassKernelResults)."""
    from concourse.bass_utils import run_bass_kernel_spmd

    xf, Wd, bc = _route_and_combine(
        inputs["x"], inputs["gate_w"], inputs["gate_b"],
        inputs["conv_w"], inputs["conv_b"],
    )
    nc = _build()
    in_maps = []
    for k in range(N_CORES):
        s = slice(k * BPC, (k + 1) * BPC)
        in_maps.append({"x": xf[s], "w": Wd[s], "bias": bc[s]})
    last_err = None
    for attempt in range(3):
        try:
            res = run_bass_kernel_spmd(
                nc, in_maps, list(range(N_CORES)),
                trace=trace, trace_cores=trace_cores,
            )
            break
        except Exception as e:  # transient NRT_EXEC_UNIT_UNRECOVERABLE flakes
            last_err = e
            time.sleep(5.0)
    else:
        raise last_err
    out = np.concatenate([r["out"] for r in res.results], axis=0)
    return out, res


def kernel(x, gate_w, gate_b, conv_w, conv_b):
    out, _ = run_sharded(
        {"x": x, "gate_w": gate_w, "gate_b": gate_b,
         "conv_w": conv_w, "conv_b": conv_b}
    )
    return out
